# revision 1
# baseline (speedup 1.0000x reference)
"""BalancedPrototypeLoss on 8 Trainium2 NeuronCores.

Strategy (data-parallel over batch, row-parallel over prototypes):
  - similarities [16384,100,10] sharded along batch across 8 cores
    (2048 samples/core = 16 tiles of 128 partitions x 1000 free).
  - per tile: smax[b,c] = max_p sims (min distance = 1 - smax) into a
    column-batched [128,16,100] buffer; one-hot(-4x) of labels vs iota on
    gpsimd. Per group of 4 tiles: own/other class stats via batched
    tensor_tensor + tensor_reduce, then per-tile PE matmuls
    rhs3^T @ onehot accumulated in PSUM giving per-class partials [3,100].
  - prototype Gram: normalize prototypes (batched rsqrt), PE-transpose to
    [D,T] layout, Gram rows for this core's 125-row slice via PE matmul,
    masked reductions for diversity / contrastive rows.
  - host combines the tiny per-core partials ([3,100] + [128,2] each) and
    evaluates the final scalar formulas in float32.
"""

import sys

_TRN_REPO = "/opt/trn_rl_repo"
if _TRN_REPO not in sys.path:
    sys.path.insert(0, _TRN_REPO)

import numpy as np

import concourse.bacc as bacc
import concourse.mybir as mybir
from concourse import tile
from concourse.masks import make_identity
from concourse.bass_utils import run_bass_kernel_spmd

fp32 = mybir.dt.float32
fp16 = mybir.dt.float16
i16 = mybir.dt.int16
USE_I16 = True
QS = 32767.0  # sims quantization scale
Alu = mybir.AluOpType
Act = mybir.ActivationFunctionType
Axis = mybir.AxisListType

B, C, P, D, T = 16384, 100, 10, 256, 1000
NCORES = 8
BC = B // NCORES     # 2048 samples per core
NT = BC // 128       # 16 batch tiles per core
GRP = 4              # tiles per reduction group
TRV = T // NCORES    # 125 prototype rows per core
NB = (T + 127) // 128  # 8 prototype blocks
MARGIN = 0.3
CLST_SCALE = 0.8
SEP_SCALE = 0.08
DIV_SCALE = 0.01
CONTRASTIVE_SCALE = 0.1

_PROGRAMS = {}


def _build(masked: bool, quant: bool):
    sdt = i16 if quant else fp32    # sims streaming dtype
    gdt = fp16 if quant else fp32   # gram / prototype dtype
    mdt = fp16 if quant else fp32   # diversity mask dtype
    nc = bacc.Bacc("TRN2", target_bir_lowering=False, debug=False,
                   num_devices=NCORES)
    sims_d = nc.dram_tensor("sims", [NT, 128, C, P], sdt, kind="ExternalInput").ap()
    oh2_d = nc.dram_tensor("oh2", [128, NT, C], fp32, kind="ExternalInput").ap()
    protos_d = nc.dram_tensor("protos", [T, D], fp32, kind="ExternalInput").ap()
    protor_d = nc.dram_tensor("protor", [128, D], fp32, kind="ExternalInput").ap()
    mdiv_d = nc.dram_tensor("maskdiv", [128, T], mdt, kind="ExternalInput").ap()
    mcon_d = None
    slot_d = None
    if masked:
        mcon_d = nc.dram_tensor("maskcon", [128, T], fp32, kind="ExternalInput").ap()
        slot_d = nc.dram_tensor("slotmask", [128, C, P], sdt, kind="ExternalInput").ap()
    outcls_d = nc.dram_tensor("out_cls", [3, C], fp32, kind="ExternalOutput").ap()
    outpr_d = nc.dram_tensor("out_pr", [128, 2], fp32, kind="ExternalOutput").ap()

    with tile.TileContext(nc) as tc:
        with (
            tc.tile_pool(name="consts", bufs=1) as consts,
            tc.tile_pool(name="simin", bufs=12) as simin,
            tc.tile_pool(name="jbuf", bufs=3) as jbuf,
            tc.tile_pool(name="cols", bufs=4) as cols,
            tc.tile_pool(name="pblk", bufs=NB + 1) as pblkp,
            tc.tile_pool(name="pp", bufs=2) as pp,
            tc.tile_pool(name="wide", bufs=2) as wide,
            tc.tile_pool(name="outp", bufs=1) as outp,
            tc.tile_pool(name="psA", bufs=1, space="PSUM") as psA,
            tc.tile_pool(name="psT", bufs=2, space="PSUM") as psT,
            tc.tile_pool(name="psG", bufs=2, space="PSUM") as psG,
        ):
            # ---------------- batch part (interleaved with proto part) ----
            SM = consts.tile([128, NT, C], sdt, tag="SM")
            OH2 = consts.tile([128, NT, C], fp32, tag="OH2")
            RHS = consts.tile([128, 3, NT], fp32, tag="RHS")
            nc.vector.memset(RHS[:, 2, :], 1.0)
            if masked:
                slot_t = consts.tile([128, C, P], sdt, tag="slot")
                nc.sync.dma_start(slot_t[:], slot_d[:])
            cls_ps = psA.tile([3, C], fp32, tag="cls")
            NG = NT // GRP

            def emit_group(g_i):
                sl = slice(g_i * GRP, (g_i + 1) * GRP)
                nc.sync.dma_start(OH2[:, sl, :], oh2_d[:, sl, :])
                for ii in range(GRP):
                    i = g_i * GRP + ii
                    st = simin.tile([128, C, P], sdt, name=f"st{i}", tag="st")
                    nc.sync.dma_start(st[:], sims_d[i])
                    if masked:
                        nc.vector.tensor_tensor(st[:], st[:], slot_t[:],
                                                op=(Alu.min if quant else Alu.add))
                    nc.vector.tensor_reduce(SM[:, i, :], st[:], axis=Axis.X,
                                            op=Alu.max)
                j2 = jbuf.tile([128, GRP, C], fp32, name=f"j2_{g_i}", tag="j2")
                nc.vector.tensor_tensor(j2[:], SM[:, sl, :], OH2[:, sl, :],
                                        op=Alu.add)
                # min over c = own - off ; max over c = other_smax
                nc.vector.tensor_reduce(RHS[:, 0, sl], j2[:], axis=Axis.X,
                                        op=Alu.min)
                nc.vector.tensor_reduce(RHS[:, 1, sl], j2[:], axis=Axis.X,
                                        op=Alu.max)
                if quant:
                    # own_min*QS = QS - s_int = -minred - (65536 - QS)
                    nc.vector.tensor_scalar(RHS[:, 0, sl], RHS[:, 0, sl],
                                            -1.0, -(65536.0 - QS), op0=Alu.mult,
                                            op1=Alu.add)
                    # sep*QS = relu(maxred - (1-margin)*QS)
                    nc.vector.tensor_scalar(RHS[:, 1, sl], RHS[:, 1, sl],
                                            -(1.0 - MARGIN) * QS, 0.0,
                                            op0=Alu.add, op1=Alu.max)
                else:
                    # own_min = 1 - own_sim = -1 - minred
                    nc.vector.tensor_scalar(RHS[:, 0, sl], RHS[:, 0, sl], -1.0,
                                            -1.0, op0=Alu.mult, op1=Alu.add)
                    # sep = relu(other_smax - (1 - margin))
                    nc.vector.tensor_scalar(RHS[:, 1, sl], RHS[:, 1, sl],
                                            -(1.0 - MARGIN), 0.0,
                                            op0=Alu.add, op1=Alu.max)
                for ii in range(GRP):
                    i = g_i * GRP + ii
                    nc.tensor.matmul(cls_ps[:], RHS[:, :, i], OH2[:, i, :],
                                     start=(i == 0), stop=(i == NT - 1))

            emit_group(0)
            emit_group(1)

            # ---- proto phase 1: loads + squared row sums ----
            mdiv_t = consts.tile([128, T], mdt, tag="mdiv")
            nc.sync.dma_start(mdiv_t[:], mdiv_d[:])
            if masked:
                mcon_t = consts.tile([128, T], fp32, tag="mcon")
                nc.sync.dma_start(mcon_t[:], mcon_d[:])
            ident = consts.tile([128, 128], gdt, tag="ident")
            make_identity(nc, ident[:])
            nhalf = consts.tile([128, 1], fp32, tag="nhalf")
            nc.vector.memset(nhalf[:], -0.5)
            pnT = [consts.tile([128, T], gdt, name=f"pnT{h}", tag=f"pnT{h}")
                   for h in (0, 1)]
            rT = [consts.tile([128, 128], gdt, name=f"rT{h}", tag=f"rT{h}")
                  for h in (0, 1)]
            SS = consts.tile([128, NB + 1], fp32, tag="SS")
            blks = []
            for b in range(NB + 1):
                blk = pblkp.tile([128, D], fp32, name=f"blk{b}", tag=f"blk{b}")
                if b < NB:
                    nrows = min(128, T - 128 * b)
                    if nrows < 128:
                        nc.vector.memset(blk[:], 0.0)
                        nc.sync.dma_start(blk[:nrows, :],
                                          protos_d[128 * b:128 * b + nrows, :])
                    else:
                        nc.sync.dma_start(blk[:], protos_d[128 * b:128 * (b + 1), :])
                else:
                    nc.sync.dma_start(blk[:], protor_d[:])
                sq = pp.tile([128, D], fp32, tag="sq")
                nc.scalar.activation(sq[:], blk[:], Act.Square,
                                     accum_out=SS[:, b:b + 1])
                blks.append(blk)

            emit_group(2)

            # ---- proto phase 2+3: norms, normalize, transpose ----
            SR = consts.tile([128, NB + 1], fp32, tag="SR")
            nc.scalar.sqrt(SR[:], SS[:])
            nc.vector.tensor_scalar_max(SR[:], SR[:], 1e-12)
            INV = consts.tile([128, NB + 1], fp32, tag="INV")
            nc.vector.reciprocal(INV[:], SR[:])
            diagss = cols.tile([128, 1], fp32, tag="diagss")
            for b in range(NB + 1):
                pnb = pp.tile([128, D], gdt, name=f"pnb{b}", tag="pnb")
                nc.scalar.activation(pnb[:], blks[b][:], Act.Copy,
                                     scale=INV[:, b:b + 1])
                for h in (0, 1):
                    tr = psT.tile([128, 128], gdt, name=f"tr{b}_{h}", tag="tr")
                    nc.tensor.transpose(tr[:], pnb[:, 128 * h:128 * (h + 1)], ident[:])
                    eng = nc.scalar if h == 0 else nc.vector
                    if b < NB:
                        nrows = min(128, T - 128 * b)
                        if h == 0:
                            nc.scalar.copy(pnT[h][:, 128 * b:128 * b + nrows],
                                           tr[:, :nrows])
                        else:
                            nc.vector.tensor_copy(pnT[h][:, 128 * b:128 * b + nrows],
                                                  tr[:, :nrows])
                    else:
                        nc.scalar.copy(rT[0][:], tr[:]) if h == 0 else \
                            nc.vector.tensor_copy(rT[1][:], tr[:])
                if b == NB and not masked:
                    # self-similarity diag[r] = sum_d pn_r[d]^2 (for conrow)
                    sqd = pp.tile([128, D], fp32, tag="sq")
                    nc.scalar.activation(sqd[:], pnb[:], Act.Square,
                                         accum_out=diagss[:])

            # ---- gram + row reductions ----
            NH = 2
            NW = T // NH
            dacc = [cols.tile([128, 1], fp32, name=f"dacc{nh}", tag=f"dacc{nh}")
                    for nh in range(NH)]
            cacc = [cols.tile([128, 1], fp32, name=f"cacc{nh}", tag=f"cacc{nh}")
                    for nh in range(NH)]
            for nh in range(NH):
                g = psG.tile([128, NW], fp32, name=f"g{nh}", tag="g")
                for k in (0, 1):
                    nc.tensor.matmul(g[:], rT[k][:], pnT[k][:, NW * nh:NW * (nh + 1)],
                                     start=(k == 0), stop=(k == 1))
                rel = wide.tile([128, NW], fp32, name=f"rel{nh}", tag="rel")
                nc.scalar.activation(rel[:], g[:], Act.Relu, bias=nhalf[:])
                junkd = wide.tile([128, NW], fp32, name=f"junkd{nh}", tag="junkd")
                nc.vector.tensor_tensor(junkd[:], rel[:],
                                        mdiv_t[:, NW * nh:NW * (nh + 1)], op=Alu.mult)
                nc.vector.tensor_reduce(dacc[nh][:], junkd[:], axis=Axis.X, op=Alu.add)
                if masked:
                    junkc = wide.tile([128, NW], fp32, name=f"junkc{nh}", tag="junkc")
                    nc.vector.tensor_tensor(junkc[:], g[:],
                                            mcon_t[:, NW * nh:NW * (nh + 1)],
                                            op=Alu.mult)
                    nc.vector.tensor_reduce(cacc[nh][:], junkc[:], axis=Axis.X,
                                            op=Alu.add)
                else:
                    nc.vector.tensor_reduce(cacc[nh][:], g[:], axis=Axis.X,
                                            op=Alu.add)
            opr = outp.tile([128, 2], fp32, tag="opr")
            nc.vector.tensor_tensor(opr[:, 0:1], dacc[0][:], dacc[1][:], op=Alu.add)
            nc.vector.tensor_tensor(opr[:, 1:2], cacc[0][:], cacc[1][:],
                                    op=Alu.add)
            if not masked:
                nc.vector.tensor_tensor(opr[:, 1:2], opr[:, 1:2], diagss[:],
                                        op=Alu.subtract)
            nc.sync.dma_start(outpr_d[:], opr[:])

            emit_group(3)

            ocl = outp.tile([3, C], fp32, tag="ocl")
            nc.vector.tensor_copy(ocl[:], cls_ps[:])
            nc.sync.dma_start(outcls_d[:], ocl[:])

    nc.compile()
    return nc


def _get_program(masked: bool):
    key = (bool(masked), USE_I16)
    if key not in _PROGRAMS:
        _PROGRAMS[key] = _build(masked, USE_I16)
    return _PROGRAMS[key]


def _numpy_fallback(similarities, labels, prototypes, proto_indices, valid_mask):
    """Pure-numpy replication of the reference (for unexpected shapes)."""
    s = similarities.astype(np.float64)
    Bx, Cx, Px = s.shape
    Tx = prototypes.shape[0]
    distances = 1.0 - s
    starts = proto_indices[:, 0]
    ends = proto_indices[:, 1]
    counts = ends - starts
    pvalid = np.arange(Px)[None, :] < counts[:, None]
    dmask = np.where(pvalid[None, :, :], distances, np.inf)
    min_all = dmask.min(axis=-1)
    own_min = min_all[np.arange(Bx), labels]
    cls_n = np.bincount(labels, minlength=Cx).astype(np.float64)
    cls_sum = np.bincount(labels, weights=own_min, minlength=Cx)
    has = cls_n > 0
    nvalid = max(int(has.sum()), 1)
    mean_c = cls_sum / np.maximum(cls_n, 1.0)
    w = 1.0 / np.sqrt(cls_n + 1e-6)
    cluster = np.where(has, w * mean_c, 0.0).sum() / nvalid * CLST_SCALE
    m2 = min_all.copy()
    m2[np.arange(Bx), labels] = np.inf
    other_min = m2.min(axis=-1)
    sep_term = np.maximum(MARGIN - other_min, 0.0)
    sep_cls = np.bincount(labels, weights=sep_term, minlength=Cx)
    sep = np.where(has, sep_cls / np.maximum(cls_n, 1.0), 0.0).sum() / nvalid * SEP_SCALE
    pr = prototypes.astype(np.float64)
    norm = np.sqrt((pr * pr).sum(-1, keepdims=True))
    pn = pr / np.maximum(norm, 1e-12)
    sim = pn @ pn.T
    proto_class = np.searchsorted(starts, np.arange(Tx), side="right") - 1
    same = proto_class[:, None] == proto_class[None, :]
    offd = ~np.eye(Tx, dtype=bool)
    pair = same & offd
    relv = np.maximum(sim - 0.5, 0.0)
    row_sum = np.where(pair, relv, 0.0).sum(1)
    cls_pair = np.bincount(proto_class, weights=row_sum, minlength=Cx)
    npairs = (counts * (counts - 1)).astype(np.float64)
    dvalid = counts > 1
    ndv = max(int(dvalid.sum()), 1)
    div = np.where(dvalid, cls_pair / np.maximum(npairs, 1.0), 0.0).sum() / ndv * DIV_SCALE
    vm = valid_mask.astype(bool)
    vpair = (vm[:, None] & vm[None, :]) & offd
    nvp = max(int(vpair.sum()), 1)
    contrast = np.where(vpair, sim, 0.0).sum() / nvp * CONTRASTIVE_SCALE
    total = cluster + sep + div + contrast
    return np.array([cluster, sep, div, contrast, total], dtype=np.float32)


def kernel(similarities, labels, prototypes, proto_indices, valid_mask,
           max_prototypes=None, **_ignored):
    similarities = np.asarray(similarities, dtype=np.float32)
    labels = np.asarray(labels)
    prototypes = np.asarray(prototypes, dtype=np.float32)
    proto_indices = np.asarray(proto_indices)
    valid_mask = np.asarray(valid_mask).astype(bool)

    if similarities.shape != (B, C, P) or prototypes.shape != (T, D):
        return _numpy_fallback(similarities, labels, prototypes,
                               proto_indices, valid_mask)

    starts = proto_indices[:, 0].astype(np.int64)
    ends = proto_indices[:, 1].astype(np.int64)
    counts = ends - starts
    pvalid = np.arange(P)[None, :] < counts[:, None]  # [C,P]
    masked = (not bool(pvalid.all())) or (not bool(np.asarray(valid_mask).all()))
    proto_class = (np.searchsorted(starts, np.arange(T), side="right") - 1)

    labels_i = labels.astype(np.int64)
    vm = valid_mask
    if USE_I16:
        sims_q = np.rint(similarities * np.float32(QS)).astype(np.int16)
    slotmask = None
    if masked:
        if USE_I16:
            slotadd = np.where(pvalid, 32767, -32768).astype(np.int16).reshape(1, C, P)
        else:
            slotadd = np.where(pvalid, 0.0, -1e30).astype(np.float32).reshape(1, C, P)
        slotmask = np.ascontiguousarray(np.broadcast_to(slotadd, (128, C, P)))

    in_maps = []
    for c in range(NCORES):
        if USE_I16:
            sl = sims_q[c * BC:(c + 1) * BC].reshape(NT, 128, C, P)
        else:
            sl = similarities[c * BC:(c + 1) * BC].reshape(NT, 128, C, P)
        lab_c = labels_i[c * BC:(c + 1) * BC].reshape(NT, 128)
        oh2 = np.zeros((128, NT, C), np.float32)
        ii, pp_ = np.meshgrid(np.arange(NT), np.arange(128), indexing="ij")
        oh2[pp_.ravel(), ii.ravel(), lab_c.ravel()] = -65536.0 if USE_I16 else -2.0
        r0 = c * TRV
        rows = np.arange(r0, r0 + 128)
        rin = rows < T
        rows_c = np.minimum(rows, T - 1)
        rcls = proto_class[rows_c]
        np_mdt = np.float16 if USE_I16 else np.float32
        md = (rcls[:, None] == proto_class[None, :]).astype(np_mdt)
        md[np.arange(128), rows_c] = 0.0  # off-diagonal
        md[~rin] = 0.0
        md[TRV:] = 0.0  # rows beyond this core's 125 handled elsewhere
        if masked:
            mc = (vm[rows_c][:, None] & vm[None, :]).astype(np.float32)
            mc[np.arange(128), rows_c] = 0.0
            mc[~rin] = 0.0
            mc[TRV:] = 0.0
        protor = np.zeros((128, D), np.float32)
        nr = min(T - r0, 128)
        protor[:nr] = prototypes[r0:r0 + nr]
        m = dict(sims=sl, oh2=oh2, protos=prototypes,
                 protor=protor, maskdiv=md)
        if masked:
            m["maskcon"] = mc
            m["slotmask"] = slotmask
        in_maps.append(m)

    nc = _get_program(masked)
    res = run_bass_kernel_spmd(nc, in_maps, core_ids=list(range(NCORES)))
    results = res.results

    oh_scale = np.float32(-1.0 / 65536.0) if USE_I16 else np.float32(-0.5)
    row_scale = np.float32(1.0 / QS) if USE_I16 else np.float32(1.0)
    cls = np.sum(np.stack([results[c]["out_cls"] for c in range(NCORES)]),
                 axis=0, dtype=np.float32) * oh_scale  # [3, C]
    cls_own = cls[0] * row_scale
    sep_cls_sum = cls[1] * row_scale
    cls_n = cls[2]
    divrow = np.concatenate([results[c]["out_pr"][:TRV, 0] for c in range(NCORES)])
    conrow = np.concatenate([results[c]["out_pr"][:TRV, 1] for c in range(NCORES)])

    f32 = np.float32
    has = cls_n > 0
    nvalid = f32(max(int(has.sum()), 1))
    mean_c = (cls_own / np.maximum(cls_n, f32(1.0))).astype(f32)
    w = (f32(1.0) / np.sqrt(cls_n + f32(1e-6))).astype(f32)
    cluster = f32(np.where(has, w * mean_c, f32(0.0)).sum(dtype=np.float32)
                  / nvalid * f32(CLST_SCALE))
    sep = f32(np.where(has, sep_cls_sum / np.maximum(cls_n, f32(1.0)), f32(0.0))
              .sum(dtype=np.float32) / nvalid * f32(SEP_SCALE))

    cls_pair = np.zeros(C, np.float32)
    np.add.at(cls_pair, proto_class, divrow)
    npairs = (counts * (counts - 1)).astype(np.float32)
    dvalid = counts > 1
    ndv = f32(max(int(dvalid.sum()), 1))
    div = f32(np.where(dvalid, cls_pair / np.maximum(npairs, f32(1.0)), f32(0.0))
              .sum(dtype=np.float32) / ndv * f32(DIV_SCALE))

    svm = int(vm.sum())
    nvp = f32(max(svm * svm - svm, 1))
    contrast = f32(conrow.sum(dtype=np.float32) / nvp * f32(CONTRASTIVE_SCALE))

    total = f32(cluster + sep + div + contrast)
    return np.array([cluster, sep, div, contrast, total], dtype=np.float32)



# revision 9
# speedup vs baseline: 1.4981x; 1.4981x over previous
"""BalancedPrototypeLoss on 8 Trainium2 NeuronCores — v2.

Strategy (data-parallel over batch, band-parallel over prototypes):
  - similarities [16384,100,10] are quantized (int8 x127 or fp16) on host and
    split into S slot-interleaved streams per 8-tile group. Stream 0 DMAs into
    SBUF (casting to fp16 in the DMA); streams 1..S-1 DMA with accum_op=max so
    the SDMA CCE units compute most of the per-class slot-max during the
    transfer. The remaining max tree runs as batched fp16 tensor_tensor ops
    (2x DVE mode) across whole 8-tile groups.
  - own/other class stats: smax plus a -K one-hot is group-max-reduced for the
    separation term; per-class sums of [smax | sep | 1] are computed by the
    TensorEngine as one-hot matmuls accumulated in PSUM ([100,102]); the
    own-class diagonal is extracted on host.
  - prototype Gram: host normalizes+transposes prototypes; each core computes
    only its 128-row slice against a 140-wide same-class band (plus one
    column against colsum(pn) giving exact Gram row sums) via 2 matmuls.
  - host combines per-core partials ([100,102] + [128,2] each) in float32.
"""

import sys

_TRN_REPO = "/opt/trn_rl_repo"
if _TRN_REPO not in sys.path:
    sys.path.insert(0, _TRN_REPO)

import numpy as np

import concourse.bacc as bacc
import concourse.mybir as mybir
from concourse import tile
from concourse.bass_utils import run_bass_kernel_spmd

fp32 = mybir.dt.float32
fp16 = mybir.dt.float16
i8 = mybir.dt.int8
Alu = mybir.AluOpType
Act = mybir.ActivationFunctionType
Axis = mybir.AxisListType

B, C, P, D, T = 16384, 100, 10, 256, 1000
NCORES = 8
BC = B // NCORES       # 2048 samples per core
NT = BC // 128         # 16 batch tiles per core
TG = 8                 # tiles per DMA/compute group
NG = NT // TG          # groups per core
TRV = T // NCORES      # 125 prototype rows per core
BAND = 140             # same-class band width (>= 130 needed)
MARGIN = 0.3
CLST_SCALE = 0.8
SEP_SCALE = 0.08
DIV_SCALE = 0.01
CONTRASTIVE_SCALE = 0.1

# --- tuning knobs ---
USE_INT8 = True        # int8 streams in HBM, DMA casts to fp16
NSTREAM = 1            # 1 (no CCE), 2, or 5 slot-interleaved CCE-max streams
Q = P // NSTREAM       # slot-groups left after CCE max

SC = 127.0 if USE_INT8 else 1.0            # quantization scale
OHV = -128.0 if USE_INT8 else -4.0         # one-hot mask/weight value
SEP_TH = (1.0 - MARGIN) * SC               # separation threshold (quant units)

_PROGRAMS = {}


def _build():
    sdt = i8 if USE_INT8 else fp16
    QC = Q * C
    nc = bacc.Bacc("TRN2", target_bir_lowering=False, debug=False,
                   num_devices=NCORES)
    sims_d = nc.dram_tensor("sims", [NSTREAM, NG, 128, TG * QC], sdt,
                            kind="ExternalInput").ap()
    ohm_d = nc.dram_tensor("ohm", [128, NT, C], sdt, kind="ExternalInput").ap()
    grhs_d = nc.dram_tensor("grhs", [128, 2, BAND + 1], fp16,
                            kind="ExternalInput").ap()
    rt2_d = nc.dram_tensor("rt2", [128, 2, 128], fp16,
                           kind="ExternalInput").ap()
    mdiv_d = nc.dram_tensor("mdiv", [128, BAND], fp16,
                            kind="ExternalInput").ap()
    outcls_d = nc.dram_tensor("out_cls", [C, C + 2], fp32,
                              kind="ExternalOutput").ap()
    outpr_d = nc.dram_tensor("out_pr", [128, 2], fp32,
                             kind="ExternalOutput").ap()

    with tile.TileContext(nc) as tc:
        with (
            tc.tile_pool(name="consts", bufs=1) as consts,
            tc.tile_pool(name="mg", bufs=2) as mgp,
            tc.tile_pool(name="scr", bufs=2) as scr,
            tc.tile_pool(name="gr", bufs=1) as grp_,
            tc.tile_pool(name="outp", bufs=1) as outp,
            tc.tile_pool(name="psA", bufs=1, space="PSUM") as psA,
            tc.tile_pool(name="psG", bufs=1, space="PSUM") as psG,
        ):
            # ---- gram inputs on HWDGE (sync) queue ----
            grhs_t = consts.tile([128, 2, BAND + 1], fp16, tag="grhs")
            rt2_t = consts.tile([128, 2, 128], fp16, tag="rt2")
            mdiv_t = consts.tile([128, BAND], fp16, tag="mdiv")
            nc.sync.dma_start(grhs_t[:], grhs_d[:])
            nc.sync.dma_start(rt2_t[:], rt2_d[:])
            nc.sync.dma_start(mdiv_t[:], mdiv_d[:])

            # ---- one-hot (cast int8 -> fp16 on SWDGE when quantized) ----
            OH = consts.tile([128, NT, C], fp16, tag="OH")
            if USE_INT8:
                nc.gpsimd.dma_start(OH[:], ohm_d[:], max_dma_last_dim=2000)
            else:
                nc.sync.dma_start(OH[:], ohm_d[:])

            RH = consts.tile([128, NT, C + 2], fp16, tag="RH")
            nc.vector.memset(RH[:, :, C + 1], 1.0)
            OM = consts.tile([128, NT], fp16, tag="OM")
            cls_ps = psA.tile([C, C + 2], fp32, tag="cls")
            nhalf = consts.tile([128, 1], fp32, tag="nhalf")
            nc.vector.memset(nhalf[:], -0.5)
            nsep = consts.tile([128, 1], fp32, tag="nsep")
            nc.vector.memset(nsep[:], -SEP_TH)

            # ---- gram: 2 matmuls + relu/mask/reduce ----
            g_ps = psG.tile([128, BAND + 1], fp32, tag="g")
            for k in (0, 1):
                nc.tensor.matmul(g_ps[:], rt2_t[:, k, :], grhs_t[:, k, :],
                                 start=(k == 0), stop=(k == 1))
            rel = grp_.tile([128, BAND], fp16, tag="rel")
            nc.scalar.activation(rel[:], g_ps[:, 0:BAND], Act.Relu,
                                 bias=nhalf[:])
            junk = grp_.tile([128, BAND], fp16, tag="junk")
            opr = outp.tile([128, 2], fp32, tag="opr")
            nc.vector.tensor_tensor(junk[:], rel[:], mdiv_t[:], op=Alu.mult)
            nc.vector.tensor_reduce(opr[:, 0:1], junk[:], axis=Axis.X,
                                    op=Alu.add)
            nc.scalar.copy(opr[:, 1:2], g_ps[:, BAND:BAND + 1])
            nc.sync.dma_start(outpr_d[:], opr[:])

            # ---- batch groups ----
            # round-robin the CCE chains across groups to keep HBM busy
            mgs = []
            for g in range(NG):
                Mg = mgp.tile([128, TG, QC], fp16, name=f"mg{g}", tag="mg")
                mgs.append(Mg)
            for s in range(NSTREAM):
                for g in range(NG):
                    op = Alu.bypass if s == 0 else Alu.max
                    if USE_INT8 or s > 0:
                        nc.gpsimd.dma_start(mgs[g][:], sims_d[s, g],
                                            accum_op=op,
                                            max_dma_last_dim=2000)
                    else:
                        nc.sync.dma_start(mgs[g][:], sims_d[s, g])

            for g in range(NG):
                Mg = mgs[g]
                sl = slice(g * TG, (g + 1) * TG)
                # max tree over the Q slot-groups (batched across TG tiles)
                if Q == 10:
                    W = scr.tile([128, TG, 500], fp16, name=f"w{g}", tag="W")
                    nc.vector.tensor_tensor(W[:], Mg[:, :, 0:500],
                                            Mg[:, :, 500:1000], op=Alu.max)
                else:
                    W = Mg
                if Q >= 5:
                    X = scr.tile([128, TG, 200], fp16, name=f"x{g}", tag="X")
                    nc.vector.tensor_tensor(X[:], W[:, :, 0:200],
                                            W[:, :, 200:400], op=Alu.max)
                    Y = scr.tile([128, TG, 100], fp16, name=f"y{g}", tag="Y")
                    nc.vector.tensor_tensor(Y[:], X[:, :, 0:100],
                                            X[:, :, 100:200], op=Alu.max)
                    nc.vector.tensor_tensor(RH[:, sl, 0:C], Y[:],
                                            W[:, :, 400:500], op=Alu.max)
                else:  # Q == 2
                    nc.vector.tensor_tensor(RH[:, sl, 0:C], Mg[:, :, 0:100],
                                            Mg[:, :, 100:200], op=Alu.max)
                # other-class max -> separation term
                J = scr.tile([128, TG, C], fp16, name=f"j{g}", tag="J")
                nc.vector.tensor_tensor(J[:], RH[:, sl, 0:C], OH[:, sl, :],
                                        op=Alu.add)
                nc.vector.tensor_reduce(OM[:, sl], J[:], axis=Axis.X,
                                        op=Alu.max)
                nc.scalar.activation(RH[:, sl, C:C + 1], OM[:, sl], Act.Relu,
                                     bias=nsep[:])
                # per-class sums via PE, accumulated in PSUM
                for t in range(TG):
                    i = g * TG + t
                    nc.tensor.matmul(cls_ps[:], OH[:, i, :], RH[:, i, :],
                                     start=(i == 0), stop=(i == NT - 1))

            ocl = outp.tile([C, C + 2], fp32, tag="ocl")
            nc.scalar.copy(ocl[:], cls_ps[:])
            nc.sync.dma_start(outcls_d[:], ocl[:])

    nc.compile()
    return nc


def _get_program():
    key = (USE_INT8, NSTREAM, TG)
    if key not in _PROGRAMS:
        _PROGRAMS[key] = _build()
    return _PROGRAMS[key]


def _numpy_fallback(similarities, labels, prototypes, proto_indices, valid_mask):
    """Pure-numpy replication of the reference (for unexpected shapes)."""
    s = similarities.astype(np.float64)
    Bx, Cx, Px = s.shape
    Tx = prototypes.shape[0]
    distances = 1.0 - s
    starts = proto_indices[:, 0]
    ends = proto_indices[:, 1]
    counts = ends - starts
    pvalid = np.arange(Px)[None, :] < counts[:, None]
    dmask = np.where(pvalid[None, :, :], distances, np.inf)
    min_all = dmask.min(axis=-1)
    own_min = min_all[np.arange(Bx), labels]
    cls_n = np.bincount(labels, minlength=Cx).astype(np.float64)
    cls_sum = np.bincount(labels, weights=own_min, minlength=Cx)
    has = cls_n > 0
    nvalid = max(int(has.sum()), 1)
    mean_c = cls_sum / np.maximum(cls_n, 1.0)
    w = 1.0 / np.sqrt(cls_n + 1e-6)
    cluster = np.where(has, w * mean_c, 0.0).sum() / nvalid * CLST_SCALE
    m2 = min_all.copy()
    m2[np.arange(Bx), labels] = np.inf
    other_min = m2.min(axis=-1)
    sep_term = np.maximum(MARGIN - other_min, 0.0)
    sep_cls = np.bincount(labels, weights=sep_term, minlength=Cx)
    sep = np.where(has, sep_cls / np.maximum(cls_n, 1.0), 0.0).sum() / nvalid * SEP_SCALE
    pr = prototypes.astype(np.float64)
    norm = np.sqrt((pr * pr).sum(-1, keepdims=True))
    pn = pr / np.maximum(norm, 1e-12)
    sim = pn @ pn.T
    proto_class = np.searchsorted(starts, np.arange(Tx), side="right") - 1
    same = proto_class[:, None] == proto_class[None, :]
    offd = ~np.eye(Tx, dtype=bool)
    pair = same & offd
    relv = np.maximum(sim - 0.5, 0.0)
    row_sum = np.where(pair, relv, 0.0).sum(1)
    cls_pair = np.bincount(proto_class, weights=row_sum, minlength=Cx)
    npairs = (counts * (counts - 1)).astype(np.float64)
    dvalid = counts > 1
    ndv = max(int(dvalid.sum()), 1)
    div = np.where(dvalid, cls_pair / np.maximum(npairs, 1.0), 0.0).sum() / ndv * DIV_SCALE
    vm = valid_mask.astype(bool)
    vpair = (vm[:, None] & vm[None, :]) & offd
    nvp = max(int(vpair.sum()), 1)
    contrast = np.where(vpair, sim, 0.0).sum() / nvp * CONTRASTIVE_SCALE
    total = cluster + sep + div + contrast
    return np.array([cluster, sep, div, contrast, total], dtype=np.float32)


def kernel(similarities, labels, prototypes, proto_indices, valid_mask,
           max_prototypes=None, **_ignored):
    similarities = np.asarray(similarities, dtype=np.float32)
    labels = np.asarray(labels)
    prototypes = np.asarray(prototypes, dtype=np.float32)
    proto_indices = np.asarray(proto_indices)
    valid_mask = np.asarray(valid_mask).astype(bool)

    starts = proto_indices[:, 0].astype(np.int64)
    ends = proto_indices[:, 1].astype(np.int64)
    counts = ends - starts

    if (similarities.shape != (B, C, P) or prototypes.shape != (T, D)
            or not bool((counts == P).all()) or not bool(valid_mask.all())):
        return _numpy_fallback(similarities, labels, prototypes,
                               proto_indices, valid_mask)

    labels_i = labels.astype(np.int64)
    QC = Q * C

    # ---- sims -> S slot-interleaved streams, grouped [S, NG, 128, TG*QC] ----
    if USE_INT8:
        sq = np.rint(similarities * np.float32(SC)).astype(np.int8)
    else:
        sq = similarities.astype(np.float16)
    # stream s, position (q, c) <- slot q*S + s
    X = sq.reshape(B, C, Q, NSTREAM).transpose(3, 0, 2, 1)  # [S, B, Q, C]
    X = np.ascontiguousarray(X).reshape(NSTREAM, B, QC)

    # ---- gram host prep ----
    nrm = np.sqrt((prototypes * prototypes).sum(-1))
    pn16 = (prototypes / np.maximum(nrm, 1e-12)[:, None]).astype(np.float16)
    pnv = pn16.astype(np.float32)
    colsum = pnv.sum(0)                           # [D]
    proto_class = np.arange(T) // P

    in_maps = []
    for c in range(NCORES):
        Xc = X[:, c * BC:(c + 1) * BC]            # [S, 2048, QC]
        sims_np = np.ascontiguousarray(
            Xc.reshape(NSTREAM, NG, TG, 128, QC).transpose(0, 1, 3, 2, 4)
        ).reshape(NSTREAM, NG, 128, TG * QC)

        lab_c = labels_i[c * BC:(c + 1) * BC].reshape(NT, 128)
        ohdt = np.int8 if USE_INT8 else np.float16
        ohm = np.zeros((128, NT, C), ohdt)
        ii, pp_ = np.meshgrid(np.arange(NT), np.arange(128), indexing="ij")
        ohm[pp_.ravel(), ii.ravel(), lab_c.ravel()] = OHV

        r0 = c * TRV
        bs = (r0 // P) * P
        rows = np.arange(r0, r0 + 128)
        rin = rows < T
        rows_c = np.minimum(rows, T - 1)
        cols = np.arange(bs, bs + BAND)
        cin = cols < T
        cols_c = np.minimum(cols, T - 1)
        # rt2[d, k, r] = pn[r0+r, 128k+d]; grhs[d, k, j] = pn[bs+j, 128k+d]
        rslice = pn16[rows_c] * rin[:, None].astype(np.float16)   # [128, D]
        rt2 = np.ascontiguousarray(
            rslice.reshape(128, 2, 128).transpose(2, 1, 0))       # [128d,2,128r]
        bslice = pn16[cols_c] * cin[:, None].astype(np.float16)   # [BAND, D]
        grhs = np.zeros((128, 2, BAND + 1), np.float16)
        grhs[:, :, 0:BAND] = bslice.reshape(BAND, 2, 128).transpose(2, 1, 0)
        grhs[:, :, BAND] = colsum.reshape(2, 128).transpose(1, 0)
        md = (proto_class[rows_c][:, None] == proto_class[cols_c][None, :])
        md &= rows_c[:, None] != cols_c[None, :]
        md &= rin[:, None] & cin[None, :]
        md[TRV:] = False
        mdiv = md.astype(np.float16)

        in_maps.append(dict(sims=sims_np, ohm=ohm, grhs=grhs, rt2=rt2,
                            mdiv=mdiv))

    nc = _get_program()
    res = run_bass_kernel_spmd(nc, in_maps, core_ids=list(range(NCORES)))
    results = res.results

    f32 = np.float32
    cls = np.sum(np.stack([results[c]["out_cls"] for c in range(NCORES)]),
                 axis=0, dtype=np.float32) / f32(OHV)   # [100, 102] true sums
    own_smax_sum = np.diag(cls[:, 0:C]).astype(f32) / f32(SC)
    sep_cls_sum = cls[:, C].astype(f32) / f32(SC)
    cls_n = cls[:, C + 1].astype(f32)

    has = cls_n > 0
    nvalid = f32(max(int(has.sum()), 1))
    own_min_sum = cls_n - own_smax_sum
    mean_c = (own_min_sum / np.maximum(cls_n, f32(1.0))).astype(f32)
    w = (f32(1.0) / np.sqrt(cls_n + f32(1e-6))).astype(f32)
    cluster = f32(np.where(has, w * mean_c, f32(0.0)).sum(dtype=np.float32)
                  / nvalid * f32(CLST_SCALE))
    sep = f32(np.where(has, sep_cls_sum / np.maximum(cls_n, f32(1.0)), f32(0.0))
              .sum(dtype=np.float32) / nvalid * f32(SEP_SCALE))

    divrow = np.concatenate([results[c]["out_pr"][:TRV, 0] for c in range(NCORES)])
    conrow = np.concatenate([results[c]["out_pr"][:TRV, 1] for c in range(NCORES)])
    cls_pair = np.zeros(C, np.float32)
    np.add.at(cls_pair, proto_class, divrow)
    npairs = (counts * (counts - 1)).astype(np.float32)
    dvalid = counts > 1
    ndv = f32(max(int(dvalid.sum()), 1))
    div = f32(np.where(dvalid, cls_pair / np.maximum(npairs, f32(1.0)), f32(0.0))
              .sum(dtype=np.float32) / ndv * f32(DIV_SCALE))

    svm = int(valid_mask.sum())
    nvp = f32(max(svm * svm - svm, 1))
    contrast = f32((conrow.sum(dtype=np.float32) - f32(T))
                   / nvp * f32(CONTRASTIVE_SCALE))

    total = f32(cluster + sep + div + contrast)
    return np.array([cluster, sep, div, contrast, total], dtype=np.float32)


# revision 10
# speedup vs baseline: 1.7105x; 1.1418x over previous
"""BalancedPrototypeLoss on 8 Trainium2 NeuronCores — v3.

Strategy (data-parallel over batch, band-parallel over prototypes):
  - similarities [16384,100,10] are cast to fp16 on host, reorganized to
    slot-major [128, NT, 10, 100] per core, and streamed over HWDGE in 4
    asymmetric groups ([2,3,5,6] tiles) so the Vector engine starts early.
  - per group, the slot-max is a batched fp16 tensor_tensor max tree
    (2x DVE mode; tensor_reduce would be capped at 1x): 1000 -> 500 -> 200
    -> 100 per tile, all tiles of a group in one instruction.
  - smax plus a -128 one-hot (int8 upload, SWDGE casts to fp16) is
    group-max-reduced for the separation term; per-class sums of
    [smax | sep | 1] are computed by the TensorEngine as one-hot matmuls
    accumulated in PSUM ([100,102]); the own-class diagonal gives the
    cluster numerator on host.
  - prototype Gram: host normalizes+transposes prototypes; each core computes
    its 128-row slice against a 140-wide same-class band plus one column
    against colsum(pn) (exact Gram row sums) via 2 matmuls.
  - host combines per-core partials ([100,102] + [128,2] each) in float32.
"""

import sys

_TRN_REPO = "/opt/trn_rl_repo"
if _TRN_REPO not in sys.path:
    sys.path.insert(0, _TRN_REPO)

import numpy as np

import concourse.bacc as bacc
import concourse.mybir as mybir
from concourse import tile
from concourse.bass_utils import run_bass_kernel_spmd

fp32 = mybir.dt.float32
fp16 = mybir.dt.float16
i8 = mybir.dt.int8
Alu = mybir.AluOpType
Act = mybir.ActivationFunctionType
Axis = mybir.AxisListType

B, C, P, D, T = 16384, 100, 10, 256, 1000
NCORES = 8
BC = B // NCORES       # 2048 samples per core
NT = BC // 128         # 16 batch tiles per core
GROUPS = [2, 3, 5, 6]  # tiles per compute group (sum = NT)
TRV = T // NCORES      # 125 prototype rows per core
BAND = 140             # same-class band width (>= 130 needed)
PC = P * C             # free size per tile
MARGIN = 0.3
CLST_SCALE = 0.8
SEP_SCALE = 0.08
DIV_SCALE = 0.01
CONTRASTIVE_SCALE = 0.1

OHV = -128.0           # one-hot mask/weight value (int8-representable)
SEP_TH = 1.0 - MARGIN  # separation threshold

_PROGRAMS = {}


def _build():
    nc = bacc.Bacc("TRN2", target_bir_lowering=False, debug=False,
                   num_devices=NCORES)
    sims_d = nc.dram_tensor("sims", [128, NT * PC], fp16,
                            kind="ExternalInput").ap()
    ohm_d = nc.dram_tensor("ohm", [128, NT, C], i8, kind="ExternalInput").ap()
    grhs_d = nc.dram_tensor("grhs", [128, 2, BAND + 1], fp16,
                            kind="ExternalInput").ap()
    rt2_d = nc.dram_tensor("rt2", [128, 2, 128], fp16,
                           kind="ExternalInput").ap()
    mdiv_d = nc.dram_tensor("mdiv", [128, BAND], fp16,
                            kind="ExternalInput").ap()
    outcls_d = nc.dram_tensor("out_cls", [C, C + 2], fp32,
                              kind="ExternalOutput").ap()
    outpr_d = nc.dram_tensor("out_pr", [128, 2], fp32,
                             kind="ExternalOutput").ap()

    with tile.TileContext(nc) as tc:
        with (
            tc.tile_pool(name="consts", bufs=1) as consts,
            tc.tile_pool(name="gr", bufs=1) as grp_,
            tc.tile_pool(name="outp", bufs=1) as outp,
            tc.tile_pool(name="psA", bufs=1, space="PSUM") as psA,
            tc.tile_pool(name="psG", bufs=1, space="PSUM") as psG,
        ):
            M = consts.tile([128, NT, PC], fp16, tag="M")
            W = consts.tile([128, NT, 500], fp16, tag="W")
            X = consts.tile([128, NT, 200], fp16, tag="X")
            Y = consts.tile([128, NT, 100], fp16, tag="Y")
            J = consts.tile([128, NT, C], fp16, tag="J")
            OH = consts.tile([128, NT, C], fp16, tag="OH")
            RH = consts.tile([128, NT, C + 2], fp16, tag="RH")
            OM = consts.tile([128, NT], fp16, tag="OM")
            nc.vector.memset(RH[:, :, C + 1], 1.0)
            nhalf = consts.tile([128, 1], fp32, tag="nhalf")
            nc.vector.memset(nhalf[:], -0.5)
            nsep = consts.tile([128, 1], fp32, tag="nsep")
            nc.vector.memset(nsep[:], -SEP_TH)
            cls_ps = psA.tile([C, C + 2], fp32, tag="cls")

            # one-hot (int8 -> fp16 cast on SWDGE; off the sims queue)
            nc.gpsimd.dma_start(OH[:], ohm_d[:], max_dma_last_dim=2000)

            # sims group DMAs on HWDGE, smallest group first
            bounds = np.cumsum([0] + GROUPS)
            for g, n in enumerate(GROUPS):
                g0, g1 = int(bounds[g]), int(bounds[g + 1])
                nc.sync.dma_start(M[:, g0:g1, :],
                                  sims_d[:, g0 * PC:g1 * PC])
                if g == 2:
                    grhs_t = consts.tile([128, 2, BAND + 1], fp16, tag="grhs")
                    rt2_t = consts.tile([128, 2, 128], fp16, tag="rt2")
                    mdiv_t = consts.tile([128, BAND], fp16, tag="mdiv")
                    nc.sync.dma_start(grhs_t[:], grhs_d[:])
                    nc.sync.dma_start(rt2_t[:], rt2_d[:])
                    nc.sync.dma_start(mdiv_t[:], mdiv_d[:])

            for g, n in enumerate(GROUPS):
                g0, g1 = int(bounds[g]), int(bounds[g + 1])
                sl = slice(g0, g1)
                # slot-max tree (slot-major: position q*C + c)
                nc.vector.tensor_tensor(W[:, sl, :], M[:, sl, 0:500],
                                        M[:, sl, 500:1000], op=Alu.max)
                nc.vector.tensor_tensor(X[:, sl, :], W[:, sl, 0:200],
                                        W[:, sl, 200:400], op=Alu.max)
                nc.vector.tensor_tensor(Y[:, sl, :], X[:, sl, 0:100],
                                        X[:, sl, 100:200], op=Alu.max)
                nc.vector.tensor_tensor(RH[:, sl, 0:C], Y[:, sl, :],
                                        W[:, sl, 400:500], op=Alu.max)
                # other-class max -> separation term
                nc.vector.tensor_tensor(J[:, sl, :], RH[:, sl, 0:C],
                                        OH[:, sl, :], op=Alu.add)
                nc.vector.tensor_reduce(OM[:, sl], J[:, sl, :], axis=Axis.X,
                                        op=Alu.max)
                nc.scalar.activation(RH[:, sl, C:C + 1], OM[:, sl], Act.Relu,
                                     bias=nsep[:])
                # per-class sums via PE, accumulated in PSUM
                for i in range(g0, g1):
                    nc.tensor.matmul(cls_ps[:], OH[:, i, :], RH[:, i, :],
                                     start=(i == 0), stop=(i == NT - 1))

                if g == 2:
                    # gram: 2 matmuls + relu/mask/reduce (inputs landed)
                    g_ps = psG.tile([128, BAND + 1], fp32, tag="g")
                    for k in (0, 1):
                        nc.tensor.matmul(g_ps[:], rt2_t[:, k, :],
                                         grhs_t[:, k, :],
                                         start=(k == 0), stop=(k == 1))
                    rel = grp_.tile([128, BAND], fp16, tag="rel")
                    nc.scalar.activation(rel[:], g_ps[:, 0:BAND], Act.Relu,
                                         bias=nhalf[:])
                    junk = grp_.tile([128, BAND], fp16, tag="junk")
                    opr = outp.tile([128, 2], fp32, tag="opr")
                    nc.vector.tensor_tensor(junk[:], rel[:], mdiv_t[:],
                                            op=Alu.mult)
                    nc.vector.tensor_reduce(opr[:, 0:1], junk[:], axis=Axis.X,
                                            op=Alu.add)
                    nc.scalar.copy(opr[:, 1:2], g_ps[:, BAND:BAND + 1])
                    nc.sync.dma_start(outpr_d[:], opr[:])

            ocl = outp.tile([C, C + 2], fp32, tag="ocl")
            nc.scalar.copy(ocl[:], cls_ps[:])
            nc.sync.dma_start(outcls_d[:], ocl[:])

    nc.compile()
    return nc


def _get_program():
    if "v3" not in _PROGRAMS:
        _PROGRAMS["v3"] = _build()
    return _PROGRAMS["v3"]


def _numpy_fallback(similarities, labels, prototypes, proto_indices, valid_mask):
    """Pure-numpy replication of the reference (for unexpected shapes)."""
    s = similarities.astype(np.float64)
    Bx, Cx, Px = s.shape
    Tx = prototypes.shape[0]
    distances = 1.0 - s
    starts = proto_indices[:, 0]
    ends = proto_indices[:, 1]
    counts = ends - starts
    pvalid = np.arange(Px)[None, :] < counts[:, None]
    dmask = np.where(pvalid[None, :, :], distances, np.inf)
    min_all = dmask.min(axis=-1)
    own_min = min_all[np.arange(Bx), labels]
    cls_n = np.bincount(labels, minlength=Cx).astype(np.float64)
    cls_sum = np.bincount(labels, weights=own_min, minlength=Cx)
    has = cls_n > 0
    nvalid = max(int(has.sum()), 1)
    mean_c = cls_sum / np.maximum(cls_n, 1.0)
    w = 1.0 / np.sqrt(cls_n + 1e-6)
    cluster = np.where(has, w * mean_c, 0.0).sum() / nvalid * CLST_SCALE
    m2 = min_all.copy()
    m2[np.arange(Bx), labels] = np.inf
    other_min = m2.min(axis=-1)
    sep_term = np.maximum(MARGIN - other_min, 0.0)
    sep_cls = np.bincount(labels, weights=sep_term, minlength=Cx)
    sep = np.where(has, sep_cls / np.maximum(cls_n, 1.0), 0.0).sum() / nvalid * SEP_SCALE
    pr = prototypes.astype(np.float64)
    norm = np.sqrt((pr * pr).sum(-1, keepdims=True))
    pn = pr / np.maximum(norm, 1e-12)
    sim = pn @ pn.T
    proto_class = np.searchsorted(starts, np.arange(Tx), side="right") - 1
    same = proto_class[:, None] == proto_class[None, :]
    offd = ~np.eye(Tx, dtype=bool)
    pair = same & offd
    relv = np.maximum(sim - 0.5, 0.0)
    row_sum = np.where(pair, relv, 0.0).sum(1)
    cls_pair = np.bincount(proto_class, weights=row_sum, minlength=Cx)
    npairs = (counts * (counts - 1)).astype(np.float64)
    dvalid = counts > 1
    ndv = max(int(dvalid.sum()), 1)
    div = np.where(dvalid, cls_pair / np.maximum(npairs, 1.0), 0.0).sum() / ndv * DIV_SCALE
    vm = valid_mask.astype(bool)
    vpair = (vm[:, None] & vm[None, :]) & offd
    nvp = max(int(vpair.sum()), 1)
    contrast = np.where(vpair, sim, 0.0).sum() / nvp * CONTRASTIVE_SCALE
    total = cluster + sep + div + contrast
    return np.array([cluster, sep, div, contrast, total], dtype=np.float32)


def kernel(similarities, labels, prototypes, proto_indices, valid_mask,
           max_prototypes=None, **_ignored):
    similarities = np.asarray(similarities, dtype=np.float32)
    labels = np.asarray(labels)
    prototypes = np.asarray(prototypes, dtype=np.float32)
    proto_indices = np.asarray(proto_indices)
    valid_mask = np.asarray(valid_mask).astype(bool)

    starts = proto_indices[:, 0].astype(np.int64)
    ends = proto_indices[:, 1].astype(np.int64)
    counts = ends - starts

    if (similarities.shape != (B, C, P) or prototypes.shape != (T, D)
            or not bool((counts == P).all()) or not bool(valid_mask.all())):
        return _numpy_fallback(similarities, labels, prototypes,
                               proto_indices, valid_mask)

    labels_i = labels.astype(np.int64)

    # ---- sims -> fp16 slot-major [B, P, C] ----
    sq = similarities.astype(np.float16)
    X = np.ascontiguousarray(sq.transpose(0, 2, 1)).reshape(B, PC)

    # ---- gram host prep ----
    nrm = np.sqrt((prototypes * prototypes).sum(-1))
    pn16 = (prototypes / np.maximum(nrm, 1e-12)[:, None]).astype(np.float16)
    colsum = pn16.astype(np.float32).sum(0)       # [D]
    proto_class = np.arange(T) // P

    in_maps = []
    for c in range(NCORES):
        Xc = X[c * BC:(c + 1) * BC]               # [2048, PC]
        sims_np = np.ascontiguousarray(
            Xc.reshape(NT, 128, PC).transpose(1, 0, 2)).reshape(128, NT * PC)

        lab_c = labels_i[c * BC:(c + 1) * BC].reshape(NT, 128)
        ohm = np.zeros((128, NT, C), np.int8)
        ii, pp_ = np.meshgrid(np.arange(NT), np.arange(128), indexing="ij")
        ohm[pp_.ravel(), ii.ravel(), lab_c.ravel()] = int(OHV)

        r0 = c * TRV
        bs = (r0 // P) * P
        rows = np.arange(r0, r0 + 128)
        rin = rows < T
        rows_c = np.minimum(rows, T - 1)
        cols = np.arange(bs, bs + BAND)
        cin = cols < T
        cols_c = np.minimum(cols, T - 1)
        # rt2[d, k, r] = pn[r0+r, 128k+d]; grhs[d, k, j] = pn[bs+j, 128k+d]
        rslice = pn16[rows_c] * rin[:, None].astype(np.float16)   # [128, D]
        rt2 = np.ascontiguousarray(
            rslice.reshape(128, 2, 128).transpose(2, 1, 0))       # [128d,2,128r]
        bslice = pn16[cols_c] * cin[:, None].astype(np.float16)   # [BAND, D]
        grhs = np.zeros((128, 2, BAND + 1), np.float16)
        grhs[:, :, 0:BAND] = bslice.reshape(BAND, 2, 128).transpose(2, 1, 0)
        grhs[:, :, BAND] = colsum.reshape(2, 128).transpose(1, 0)
        md = (proto_class[rows_c][:, None] == proto_class[cols_c][None, :])
        md &= rows_c[:, None] != cols_c[None, :]
        md &= rin[:, None] & cin[None, :]
        md[TRV:] = False
        mdiv = md.astype(np.float16)

        in_maps.append(dict(sims=sims_np, ohm=ohm, grhs=grhs, rt2=rt2,
                            mdiv=mdiv))

    nc = _get_program()
    res = run_bass_kernel_spmd(nc, in_maps, core_ids=list(range(NCORES)))
    results = res.results

    f32 = np.float32
    cls = np.sum(np.stack([results[c]["out_cls"] for c in range(NCORES)]),
                 axis=0, dtype=np.float32) / f32(OHV)   # [100, 102] true sums
    own_smax_sum = np.diag(cls[:, 0:C]).astype(f32)
    sep_cls_sum = cls[:, C].astype(f32)
    cls_n = cls[:, C + 1].astype(f32)

    has = cls_n > 0
    nvalid = f32(max(int(has.sum()), 1))
    own_min_sum = cls_n - own_smax_sum
    mean_c = (own_min_sum / np.maximum(cls_n, f32(1.0))).astype(f32)
    w = (f32(1.0) / np.sqrt(cls_n + f32(1e-6))).astype(f32)
    cluster = f32(np.where(has, w * mean_c, f32(0.0)).sum(dtype=np.float32)
                  / nvalid * f32(CLST_SCALE))
    sep = f32(np.where(has, sep_cls_sum / np.maximum(cls_n, f32(1.0)), f32(0.0))
              .sum(dtype=np.float32) / nvalid * f32(SEP_SCALE))

    divrow = np.concatenate([results[c]["out_pr"][:TRV, 0] for c in range(NCORES)])
    conrow = np.concatenate([results[c]["out_pr"][:TRV, 1] for c in range(NCORES)])
    cls_pair = np.zeros(C, np.float32)
    np.add.at(cls_pair, proto_class, divrow)
    npairs = (counts * (counts - 1)).astype(np.float32)
    dvalid = counts > 1
    ndv = f32(max(int(dvalid.sum()), 1))
    div = f32(np.where(dvalid, cls_pair / np.maximum(npairs, f32(1.0)), f32(0.0))
              .sum(dtype=np.float32) / ndv * f32(DIV_SCALE))

    svm = int(valid_mask.sum())
    nvp = f32(max(svm * svm - svm, 1))
    contrast = f32((conrow.sum(dtype=np.float32) - f32(T))
                   / nvp * f32(CONTRASTIVE_SCALE))

    total = f32(cluster + sep + div + contrast)
    return np.array([cluster, sep, div, contrast, total], dtype=np.float32)
